# revision 16
# baseline (speedup 1.0000x reference)
"""Mamba2 mixer kernel for 8 trn2 NeuronCores, tensor-parallel over heads.

Each core k handles heads 8k..8k+7 (d_inner channels 512k..512k+512):
  - in_proj slice (z, x, dt columns; B/C computed redundantly on all cores)
  - causal depthwise conv + silu over its x channels + B/C
  - chunked SSD scan for its 8 heads (internal chunk size 128)
  - gated output y * silu(z); RMSNorm over the full 4096 channels uses an
    AllReduce of per-token partial sums of squares.
Host only reshapes/slices inputs and concatenates the 8 output slices.
"""

import numpy as np

import concourse.bass as bass
import concourse.tile as tile
from concourse import bacc, mybir
from concourse.bass_utils import run_bass_kernel_spmd

F32 = mybir.dt.float32
F32R = mybir.dt.float32r
AF = mybir.ActivationFunctionType
ALU = mybir.AluOpType

# dims
B_, L_, DM = 2, 2048, 2048
DS, DC, HD, NG = 128, 4, 64, 1
DI = 2 * DM              # 4096
NH = DI // HD            # 64
NCORE = 8
HPC = NH // NCORE        # 8 heads per core
XC = DI // NCORE         # 512 x/z channels per core
T = B_ * L_              # 4096 tokens
SEG = 256                # tokens per segment (in_proj/conv granularity)
NSEG = T // SEG          # 16
CHK = 128                # internal SSD chunk size
NKB = DM // 128          # 16 contraction blocks
NBLK = T // 128          # 32 token blocks
EPS = 1e-5
NEG = -1e30


def _build_nc():
    nc = bacc.Bacc("TRN2", target_bir_lowering=False, num_devices=NCORE)

    uT = nc.dram_tensor("uT", [DM, T], F32R, kind="ExternalInput")
    w_xbc = nc.dram_tensor("w_xbc", [DM, 776], F32R, kind="ExternalInput")
    w_z = nc.dram_tensor("w_z", [DM, XC], F32R, kind="ExternalInput")
    convw = nc.dram_tensor("convw", [128, 24], F32, kind="ExternalInput")
    convb = nc.dram_tensor("convb", [128, 6], F32, kind="ExternalInput")
    dtb = nc.dram_tensor("dtb", [HPC, 1], F32, kind="ExternalInput")
    alog = nc.dram_tensor("alog", [HPC, 1], F32, kind="ExternalInput")
    dsk = nc.dram_tensor("dsk", [HPC, 1], F32, kind="ExternalInput")
    nrmw = nc.dram_tensor("nrmw", [HPC, XC], F32, kind="ExternalInput")
    onehot = nc.dram_tensor("onehot", [HPC, 9 * 128], F32, kind="ExternalInput")
    maskadd = nc.dram_tensor("maskadd", [128, CHK], F32, kind="ExternalInput")
    ident = nc.dram_tensor("ident", [128, 128], F32, kind="ExternalInput")

    out = nc.dram_tensor("out", [T, XC], F32, kind="ExternalOutput")

    with tile.TileContext(nc) as tc:
        with (
            tc.tile_pool(name="wpool", bufs=1) as wpool,
            tc.tile_pool(name="cpool", bufs=1) as cpool,
            tc.tile_pool(name="upool", bufs=2) as upool,
            tc.tile_pool(name="xpool", bufs=2) as xpool,
            tc.tile_pool(name="ypool", bufs=2) as ypool,
            tc.tile_pool(name="wk2", bufs=2) as wk2,
            tc.tile_pool(name="w512", bufs=3) as w512,
            tc.tile_pool(name="hwork", bufs=3) as hwork,
            tc.tile_pool(name="rpool", bufs=2) as rpool,
            tc.tile_pool(name="ps_big", bufs=2, space="PSUM") as ps_big,
            tc.tile_pool(name="ps_bc", bufs=1, space="PSUM") as ps_bc,
            tc.tile_pool(name="ps_sm", bufs=2, space="PSUM") as ps_sm,
            tc.tile_pool(name="ps_ya", bufs=3, space="PSUM") as ps_ya,
            tc.tile_pool(name="dram", bufs=1, space="DRAM") as dram,
        ):
            # ---------------- constants / weights -------------------------
            wx_sb = wpool.tile([128, NKB, 776], F32R)
            nc.sync.dma_start(wx_sb[:], w_xbc.rearrange("(o p) c -> p o c", p=128))
            wz_sb = wpool.tile([128, NKB, XC], F32R)
            nc.sync.dma_start(wz_sb[:], w_z.rearrange("(o p) c -> p o c", p=128))
            cw_sb = cpool.tile([128, 24], F32)
            nc.sync.dma_start(cw_sb[:], convw[:])
            cb_sb = cpool.tile([128, 6], F32)
            nc.sync.dma_start(cb_sb[:], convb[:])
            dtb_sb = cpool.tile([HPC, 1], F32)
            nc.sync.dma_start(dtb_sb[:], dtb[:])
            alog_sb = cpool.tile([HPC, 1], F32)
            nc.sync.dma_start(alog_sb[:], alog[:])
            dsk_sb = cpool.tile([HPC, 1], F32)
            nc.sync.dma_start(dsk_sb[:], dsk[:])
            oh_sb = cpool.tile([HPC, 9 * 128], F32)
            nc.sync.dma_start(oh_sb[:], onehot[:])
            mk_sb = cpool.tile([128, CHK], F32)
            nc.sync.dma_start(mk_sb[:], maskadd[:])
            id_sb = cpool.tile([128, 128], F32)
            nc.sync.dma_start(id_sb[:], ident[:])
            ones8 = oh_sb[:, 8 * 128 : 9 * 128]          # [8,128] all ones
            nrm8 = cpool.tile([HPC, XC], F32)
            nc.sync.dma_start(nrm8[:], nrmw[:])

            # A = -exp(A_log)
            a_sb = cpool.tile([HPC, 1], F32)
            nc.scalar.activation(a_sb[:], alog_sb[:], AF.Exp)
            nc.vector.tensor_scalar_mul(a_sb[:], a_sb[:], -1.0)

            # broadcast D_skip and norm_w to 128 partitions via ones matmul
            dgD = cpool.tile([HPC, HPC], F32)
            nc.vector.tensor_scalar_mul(dgD[:], id_sb[0:HPC, 0:HPC], dsk_sb[:, 0:1])
            pD = ps_sm.tile([128, 256], F32, tag="sm")
            nc.tensor.matmul(pD[:, 0:HPC], ones8, dgD[:], start=True, stop=True)
            D_bc = cpool.tile([128, HPC], F32)
            nc.vector.tensor_copy(D_bc[:], pD[:, 0:HPC])

            nrm_bc = cpool.tile([128, XC], F32)
            for q in range(2):
                pN = ps_sm.tile([128, 256], F32, tag="sm")
                nc.tensor.matmul(
                    pN[:],
                    oh_sb[:, 0:128],
                    nrm8[:, q * 256 : (q + 1) * 256],
                    start=True,
                    stop=True,
                )
                nc.vector.tensor_copy(nrm_bc[:, q * 256 : (q + 1) * 256], pN[:])

            # DRAM scratch
            yg_dram = dram.tile([NBLK, 128, XC], F32)
            cc_in = dram.tile([128, NBLK], F32)
            cc_out = dram.tile([128, NBLK], F32)

            partial = cpool.tile([128, NBLK], F32)

            # persistent SSD carry state (all heads): [n, h, p]
            R_prev = None
            prev_xsrc = None

            # ---------------- main loop over segments ---------------------
            for seg in range(NSEG):
                seg_first = seg % (NSEG // B_) == 0
                # u^T tiles for this segment
                ut = upool.tile([128, NKB, SEG], F32R, tag="ut")
                nc.sync.dma_start(
                    ut[:],
                    uT.rearrange("(o p) t -> p o t", p=128)[
                        :, :, seg * SEG : (seg + 1) * SEG
                    ],
                )

                # ---- in_proj orientation 1: [ch, t] for x/B/C/dt --------
                xsrc = xpool.tile([128, 6, SEG + 3], F32, tag="xsrc")
                for cb in range(6):
                    p1 = ps_big.tile([128, XC], F32, tag="big")
                    for kb in range(NKB):
                        nc.tensor.matmul(
                            p1[:, 0:SEG],
                            wx_sb[:, kb, cb * 128 : (cb + 1) * 128],
                            ut[:, kb, :],
                            start=(kb == 0),
                            stop=(kb == NKB - 1),
                        )
                    nc.scalar.copy(xsrc[:, cb, 3 : SEG + 3], p1[:, 0:SEG])
                # dt columns (8 wide)
                pdt = ps_sm.tile([128, 256], F32, tag="sm")
                for kb in range(NKB):
                    nc.tensor.matmul(
                        pdt[0:HPC, :],
                        wx_sb[:, kb, 768:776],
                        ut[:, kb, :],
                        start=(kb == 0),
                        stop=(kb == NKB - 1),
                    )
                dtraw = wk2.tile([HPC, SEG], F32, tag="dtraw")
                nc.scalar.copy(dtraw[:], pdt[0:HPC, :])

                # conv halo
                if seg_first:
                    nc.vector.memset(xsrc[:, :, 0:3], 0.0)
                else:
                    nc.vector.tensor_copy(
                        xsrc[:, :, 0:3], prev_xsrc[:, :, SEG : SEG + 3]
                    )
                prev_xsrc = xsrc

                # ---- in_proj orientation 2: z [t, ch] + silu ------------
                zs = ypool.tile([128, 2, XC], F32, tag="zsil")
                for tb in range(2):
                    pz = ps_big.tile([128, XC], F32, tag="big")
                    for kb in range(NKB):
                        nc.tensor.matmul(
                            pz[:],
                            ut[:, kb, tb * 128 : (tb + 1) * 128],
                            wz_sb[:, kb, :],
                            start=(kb == 0),
                            stop=(kb == NKB - 1),
                        )
                    ez = w512.tile([128, XC], F32, tag="w512")
                    nc.scalar.activation(ez[:], pz[:], AF.Exp, scale=-1.0)
                    nc.vector.tensor_scalar_add(ez[:], ez[:], 1.0)
                    nc.vector.reciprocal_approx_fast(ez[:], ez[:])
                    nc.vector.tensor_mul(zs[:, tb, :], pz[:], ez[:])

                # ---- conv + silu ----------------------------------------
                xc = xpool.tile([128, 4, SEG], F32, tag="xconv")    # x channels
                bcv = xpool.tile([128, 2, SEG], F32, tag="bcconv")  # B, C
                for cb in range(6):
                    acc = wk2.tile([128, SEG], F32, tag="acc")
                    nc.vector.tensor_scalar(
                        acc[:],
                        xsrc[:, cb, 0:SEG],
                        cw_sb[:, 4 * cb : 4 * cb + 1],
                        cb_sb[:, cb : cb + 1],
                        ALU.mult,
                        ALU.add,
                    )
                    for k in range(1, 4):
                        nc.vector.scalar_tensor_tensor(
                            acc[:],
                            xsrc[:, cb, k : k + SEG],
                            cw_sb[:, 4 * cb + k : 4 * cb + k + 1],
                            acc[:],
                            ALU.mult,
                            ALU.add,
                        )
                    ec = wk2.tile([128, SEG], F32, tag="ec")
                    nc.scalar.activation(ec[:], acc[:], AF.Exp, scale=-1.0)
                    nc.vector.tensor_scalar_add(ec[:], ec[:], 1.0)
                    nc.vector.reciprocal_approx_fast(ec[:], ec[:])
                    dst = xc[:, cb, :] if cb < 4 else bcv[:, cb - 4, :]
                    nc.vector.tensor_mul(dst, acc[:], ec[:])

                # ---- dt -> softplus -> dA -> Acs (per 128-chunk) --------
                dd = wk2.tile([32 + HPC, SEG], F32, tag="dd")
                acs = dd[0:HPC, :]
                dsp = dd[32 : 32 + HPC, :]
                nc.scalar.activation(dsp, dtraw[:], AF.Exp, bias=dtb_sb[:, 0:1])
                nc.vector.tensor_scalar_add(dsp, dsp, 1.0)
                nc.scalar.activation(dsp, dsp, AF.Ln)
                dA = wk2.tile([HPC, SEG], F32, tag="dA")
                nc.vector.tensor_scalar_mul(dA[:], dsp, a_sb[:, 0:1])
                for cc in range(2):
                    nc.vector.tensor_tensor_scan(
                        acs[:, cc * CHK : (cc + 1) * CHK],
                        dA[:, cc * CHK : (cc + 1) * CHK],
                        dA[:, cc * CHK : (cc + 1) * CHK],
                        0.0,
                        ALU.add,
                        ALU.bypass,
                    )

                # ---- per 128-token chunk --------------------------------
                y_sb = ypool.tile([128, 2, XC], F32, tag="y")
                for cc in range(2):
                    first = seg_first and cc == 0
                    t0 = cc * CHK
                    tsl = slice(t0, t0 + CHK)

                    # transpose [dsp; acs] chunk -> ddT [128, 16]
                    ddT = wk2.tile([128, 32 + HPC], F32, tag=f"ddT{cc}")
                    ptr = ps_sm.tile([128, 256], F32, tag="sm")
                    nc.tensor.transpose(
                        ptr[:, 0 : 32 + HPC],
                        dd[:, tsl],
                        id_sb[0 : 32 + HPC, 0 : 32 + HPC],
                    )
                    nc.vector.tensor_copy(ddT[:], ptr[:, 0 : 32 + HPC])
                    acsT = ddT[:, 0:HPC]
                    dtT = ddT[:, 32 : 32 + HPC]
                    eaT = wk2.tile([128, HPC], F32, tag=f"eaT{cc}")
                    nc.scalar.activation(eaT[:], acsT, AF.Exp)

                    # exp(Alast) broadcast [128, 8]
                    eal8 = wk2.tile([HPC, 1], F32, tag="eal8")
                    nc.scalar.activation(
                        eal8[:], acs[:, t0 + CHK - 1 : t0 + CHK], AF.Exp
                    )
                    dg = wk2.tile([HPC, HPC], F32, tag="dg")
                    nc.vector.tensor_scalar_mul(
                        dg[:], id_sb[0:HPC, 0:HPC], eal8[:, 0:1]
                    )
                    pe_ = ps_sm.tile([128, 256], F32, tag="sm")
                    nc.tensor.matmul(
                        pe_[:, 0:HPC], ones8, dg[:], start=True, stop=True
                    )
                    eal_bc = wk2.tile([128, HPC], F32, tag=f"ealbc{cc}")
                    nc.vector.tensor_copy(eal_bc[:], pe_[:, 0:HPC])

                    # B transpose [s, n]
                    BT = wk2.tile([128, 128], F32, tag=f"BT{cc}")
                    pb = ps_sm.tile([128, 256], F32, tag="sm")
                    nc.tensor.transpose(pb[:, 0:128], bcv[:, 0, tsl], id_sb[:])
                    nc.vector.tensor_copy(BT[:], pb[:, 0:128])

                    # BC^T [s, t] -> SBUF copy (cheap reads for all heads)
                    pbc = ps_bc.tile([128, CHK], F32, tag="bc")
                    nc.tensor.matmul(
                        pbc[:], bcv[:, 0, tsl], bcv[:, 1, tsl], start=True, stop=True
                    )
                    bcm = wk2.tile([128, CHK], F32, tag=f"bcm{cc}")
                    nc.vector.tensor_copy(bcm[:], pbc[:])

                    # x transpose -> xT [s, 512]
                    xT = xpool.tile([128, XC], F32, tag=f"xT{cc}")
                    for xp_ in range(2):
                        px = ps_sm.tile([128, 256], F32, tag="sm")
                        for j in range(2):
                            nc.tensor.transpose(
                                px[:, j * 128 : (j + 1) * 128],
                                xc[:, xp_ * 2 + j, tsl],
                                id_sb[:],
                            )
                        nc.vector.tensor_copy(
                            xT[:, xp_ * 256 : (xp_ + 1) * 256], px[:]
                        )

                    py_all = ps_ya.tile([128, XC], F32, tag="ya")
                    ps_all = ps_ya.tile([128, XC], F32, tag="ya")
                    po_all = None if first else ps_ya.tile([128, XC], F32, tag="ya")
                    for h in range(HPC):
                        hsl = slice(h * HD, (h + 1) * HD)
                        # broadcast Acs row h (exact fp32)
                        pab = ps_sm.tile([128, 256], F32, tag="sm")
                        nc.tensor.matmul(
                            pab[:, 0:CHK],
                            oh_sb[:, h * 128 : (h + 1) * 128],
                            acs[:, tsl],
                            start=True,
                            stop=True,
                        )
                        est = hwork.tile([128, CHK], F32, tag="est")
                        nc.vector.scalar_tensor_tensor(
                            est[:],
                            pab[:, 0:CHK],
                            acsT[:, h : h + 1],
                            mk_sb[:],
                            ALU.subtract,
                            ALU.add,
                        )
                        nc.scalar.activation(est[:], est[:], AF.Exp)
                        # M[s,t] = est * dt[s] * BC[s,t]
                        M = hwork.tile([128, CHK], F32, tag="M")
                        nc.vector.scalar_tensor_tensor(
                            M[:],
                            est[:],
                            dtT[:, h : h + 1],
                            bcm[:],
                            ALU.mult,
                            ALU.mult,
                        )
                        # Yd
                        nc.tensor.matmul(
                            py_all[:, hsl], M[:], xT[:, hsl], start=True, stop=True
                        )
                        # x scaled by decay*dt for the state matmul
                        xs = hwork.tile([128, HD], F32, tag="xs")
                        nc.gpsimd.tensor_scalar(
                            xs[:],
                            xT[:, hsl],
                            est[:, CHK - 1 : CHK],
                            dtT[:, h : h + 1],
                            ALU.mult,
                            ALU.mult,
                        )
                        nc.tensor.matmul(
                            ps_all[:, hsl], BT[:], xs[:], start=True, stop=True
                        )
                        # Yo from carried state (before R update)
                        if not first:
                            nc.tensor.matmul(
                                po_all[:, hsl],
                                bcv[:, 1, tsl],
                                R_prev[:, h, :],
                                start=True,
                                stop=True,
                            )
                    # ---- merged y assembly + R update over all heads ----
                    ysl = y_sb[:, cc, :]
                    t1 = hwork.tile([128, XC], F32, tag="t1")
                    nc.gpsimd.tensor_tensor(
                        t1[:].rearrange("p (h d) -> p h d", h=HPC),
                        xT[:].rearrange("p (h d) -> p h d", h=HPC),
                        D_bc[:, :, None].to_broadcast((128, HPC, HD)),
                        ALU.mult,
                    )
                    nc.vector.tensor_tensor(ysl, t1[:], py_all[:], ALU.add)
                    if not first:
                        t2 = hwork.tile([128, XC], F32, tag="t1")
                        nc.vector.tensor_tensor(
                            t2[:].rearrange("p (h d) -> p h d", h=HPC),
                            po_all[:].rearrange("p (h d) -> p h d", h=HPC),
                            eaT[:, :, None].to_broadcast((128, HPC, HD)),
                            ALU.mult,
                        )
                        nc.vector.tensor_tensor(ysl, ysl, t2[:], ALU.add)
                    Rn = rpool.tile([128, HPC, HD], F32, tag="R")
                    if first:
                        nc.vector.tensor_copy(Rn[:], ps_all[:].rearrange("p (h d) -> p h d", h=HPC))
                    else:
                        t3 = hwork.tile([128, XC], F32, tag="t1")
                        nc.gpsimd.tensor_tensor(
                            t3[:].rearrange("p (h d) -> p h d", h=HPC),
                            R_prev[:],
                            eal_bc[:, :, None].to_broadcast((128, HPC, HD)),
                            ALU.mult,
                        )
                        nc.vector.tensor_tensor(
                            Rn[:],
                            t3[:].rearrange("p (h d) -> p h d", h=HPC),
                            ps_all[:].rearrange("p (h d) -> p h d", h=HPC),
                            ALU.add,
                        )
                    R_prev = Rn

                # ---- gate + partial sumsq + spill yg --------------------
                for tb in range(2):
                    yg = w512.tile([128, XC], F32, tag="w512")
                    nc.gpsimd.tensor_mul(yg[:], y_sb[:, tb, :], zs[:, tb, :])
                    sq = w512.tile([128, XC], F32, tag="w512")
                    nc.scalar.activation(
                        sq[:],
                        yg[:],
                        AF.Square,
                        accum_out=partial[:, seg * 2 + tb : seg * 2 + tb + 1],
                    )
                    nc.gpsimd.dma_start(yg_dram[seg * 2 + tb], yg[:])

            # ---------------- allreduce + normalize -----------------------
            nc.sync.dma_start(cc_in[:], partial[:])
            nc.gpsimd.collective_compute(
                "AllReduce",
                ALU.add,
                replica_groups=[list(range(NCORE))],
                ins=[cc_in.opt()],
                outs=[cc_out.opt()],
            )
            tot = cpool.tile([128, NBLK], F32)
            nc.sync.dma_start(tot[:], cc_out[:])
            epsc = cpool.tile([128, 1], F32)
            nc.vector.memset(epsc[:], EPS)
            scl = cpool.tile([128, NBLK], F32)
            nc.scalar.activation(scl[:], tot[:], AF.Ln, bias=epsc[:], scale=1.0 / DI)
            nc.scalar.activation(scl[:], scl[:], AF.Exp, scale=-0.5)

            for blk in range(NBLK):
                ygr = w512.tile([128, XC], F32, tag="w512")
                nc.gpsimd.dma_start(ygr[:], yg_dram[blk])
                nc.gpsimd.tensor_scalar_mul(ygr[:], ygr[:], scl[:, blk : blk + 1])
                nc.gpsimd.tensor_mul(ygr[:], ygr[:], nrm_bc[:])
                nc.sync.dma_start(out[blk * 128 : (blk + 1) * 128, :], ygr[:])

    nc.compile()
    return nc


_NC = None


def _host_inputs(u, w_in, conv_w, conv_b, dt_bias, A_log, D_skip, norm_w):
    """Build the 8 per-core input dicts."""
    u2 = np.ascontiguousarray(u.reshape(T, DM).T)          # [DM, T]
    onehot = np.zeros((HPC, 9 * 128), np.float32)
    for h in range(HPC):
        onehot[h, h * 128 : (h + 1) * 128] = 1.0
    onehot[:, 8 * 128 : 9 * 128] = 1.0                     # all-ones block
    mask = np.zeros((128, CHK), np.float32)
    for p in range(128):
        mask[p, 0:p] = NEG                                 # t < s masked
    ident = np.eye(128, dtype=np.float32)

    ins = []
    for k in range(NCORE):
        xcols = np.arange(DI + k * XC, DI + (k + 1) * XC)
        bcols = np.arange(2 * DI, 2 * DI + 2 * DS)
        dtcols = np.arange(
            2 * DI + 2 * DS + k * HPC, 2 * DI + 2 * DS + (k + 1) * HPC
        )
        w_xbc = np.ascontiguousarray(
            np.concatenate([w_in[:, xcols], w_in[:, bcols], w_in[:, dtcols]], 1)
        )                                                   # [DM, 776]
        w_z = np.ascontiguousarray(w_in[:, k * XC : (k + 1) * XC])
        chans = np.concatenate(
            [np.arange(k * XC, (k + 1) * XC), np.arange(DI, DI + 2 * DS)]
        )
        cw = conv_w[chans]                                  # [768, 4]
        cb = conv_b[chans]
        cw_p = np.zeros((128, 24), np.float32)
        cb_p = np.zeros((128, 6), np.float32)
        for cbk in range(6):
            cw_p[:, 4 * cbk : 4 * cbk + 4] = cw[cbk * 128 : (cbk + 1) * 128]
            cb_p[:, cbk] = cb[cbk * 128 : (cbk + 1) * 128]
        ins.append(
            dict(
                uT=u2,
                w_xbc=w_xbc,
                w_z=w_z,
                convw=cw_p,
                convb=cb_p,
                dtb=np.ascontiguousarray(dt_bias[k * HPC : (k + 1) * HPC, None]),
                alog=np.ascontiguousarray(A_log[k * HPC : (k + 1) * HPC, None]),
                dsk=np.ascontiguousarray(D_skip[k * HPC : (k + 1) * HPC, None]),
                nrmw=np.ascontiguousarray(
                    np.repeat(norm_w[None, k * XC : (k + 1) * XC], HPC, 0)
                ),
                onehot=onehot,
                maskadd=mask,
                ident=ident,
            )
        )
    return ins


def kernel(u, w_in, conv_w, conv_b, dt_bias, A_log, D_skip, norm_w):
    global _NC
    u = np.asarray(u, np.float32)
    w_in = np.asarray(w_in, np.float32)
    conv_w = np.asarray(conv_w, np.float32)
    conv_b = np.asarray(conv_b, np.float32)
    dt_bias = np.asarray(dt_bias, np.float32)
    A_log = np.asarray(A_log, np.float32)
    D_skip = np.asarray(D_skip, np.float32)
    norm_w = np.asarray(norm_w, np.float32)

    if _NC is None:
        _NC = _build_nc()
    ins = _host_inputs(u, w_in, conv_w, conv_b, dt_bias, A_log, D_skip, norm_w)
    res = run_bass_kernel_spmd(_NC, ins, core_ids=list(range(NCORE)))
    full = np.concatenate([res.results[k]["out"] for k in range(NCORE)], axis=1)
    return full.reshape(B_, L_, DI)
